# revision 3
# baseline (speedup 1.0000x reference)
"""Trainium2 kernel for nn_Attention_57595511439927 (sparse_attention).

Sharding: 8 NeuronCores = 4 images x 2 branches.
  - devices 0-3: branch 0 (global channel/transposed attention) data-parallel over b
  - devices 4-7: branch 1 (shifted-window cosine attention)      data-parallel over b
Both branches are independent per image, so no collectives are needed.
Host does only slicing / reassembly / final add glue.
"""

import numpy as np
import jax
import jax.numpy as jnp
from jax.sharding import Mesh, NamedSharding, PartitionSpec as P

WS = 8
SS = WS // 2
DIM, HEADS = 192, 6
B, H, W = 4, 256, 256


def window_partition(x, ws):
    b, h, w, c = x.shape
    x = x.reshape(b, h // ws, ws, w // ws, ws, c)
    return x.transpose(0, 1, 3, 2, 4, 5).reshape(-1, ws, ws, c)


def window_reverse(win, ws, h, w):
    b = win.shape[0] // ((h // ws) * (w // ws))
    x = win.reshape(b, h // ws, w // ws, ws, ws, -1)
    return x.transpose(0, 1, 3, 2, 4, 5).reshape(b, h, w, -1)


def rel_pos_index(ws):
    coords = np.stack(np.meshgrid(np.arange(ws), np.arange(ws), indexing="ij"))
    cf = coords.reshape(2, -1)
    rel = (cf[:, :, None] - cf[:, None, :]).transpose(1, 2, 0)
    rel[:, :, 0] += ws - 1
    rel[:, :, 1] += ws - 1
    rel[:, :, 0] *= 2 * ws - 1
    return rel.sum(-1)


def calc_mask_np(h, w, ws, ss):
    img = np.zeros((1, h, w, 1), np.float32)
    cnt = 0
    sl = (slice(0, -ws), slice(-ws, -ss), slice(-ss, None))
    for hs in sl:
        for wsl in sl:
            img[:, hs, wsl, :] = cnt
            cnt += 1
    mw = (
        img.reshape(1, h // ws, ws, w // ws, ws, 1)
        .transpose(0, 1, 3, 2, 4, 5)
        .reshape(-1, ws * ws)
    )
    diff = mw[:, None, :] - mw[:, :, None]
    return np.where(diff != 0, -100.0, 0.0).astype(np.float32)


def conv1x1(x, w):
    return jnp.einsum("bchw,oc->bohw", x, w)


def dwconv3(x, w):
    # depthwise 3x3 stride 1 pad 1 implemented as 9 shifted adds (XLA friendly)
    b, c, h, ww = x.shape
    xp = jnp.pad(x, ((0, 0), (0, 0), (1, 1), (1, 1)))
    out = jnp.zeros_like(x)
    for i in range(3):
        for j in range(3):
            out = out + w[:, 0, i, j][None, :, None, None] * xp[:, :, i : i + h, j : j + ww]
    return out


def l2norm(x, axis):
    return x / jnp.maximum(jnp.linalg.norm(x, axis=axis, keepdims=True), 1e-12)


def _branch0(x, Wq0, Wqdw0, Wkv0, Wkvdw0, Wproj0, temp0):
    b, c, h, w = x.shape
    heads = HEADS
    ch = c // heads
    q = dwconv3(conv1x1(x, Wq0), Wqdw0)
    kv = dwconv3(conv1x1(x, Wkv0), Wkvdw0)
    q = l2norm(q.reshape(b, heads, ch, h * w), -1)
    k = l2norm(kv[:, :c].reshape(b, heads, ch, h * w), -1)
    v = kv[:, c:].reshape(b, heads, ch, h * w)
    attn = jax.nn.softmax(jnp.einsum("bhcn,bhdn->bhcd", q, k) * temp0, axis=-1)
    out0 = jnp.einsum("bhcd,bhdn->bhcn", attn, v)
    out0 = out0.transpose(0, 3, 1, 2).reshape(b, h * w, c) @ Wproj0.T
    return out0.reshape(b, h, w, c).transpose(0, 3, 1, 2)


def _branch1(x, Wq1, Wqdw1, Wkv1, Wkvdw1, Wproj1, temp1, rpb, mask, Wds):
    b, c, h, w = x.shape
    heads = HEADS
    ch = c // heads
    N = WS * WS
    h2, w2 = h // 2, w // 2
    xds = x.reshape(b, c, h2, 2, w2, 2).mean(axis=(3, 5))
    xds = xds + conv1x1(xds, Wds)
    qw = window_partition(x.transpose(0, 2, 3, 1), WS).reshape(-1, N, c) @ Wq1.T
    q = window_reverse(qw.reshape(-1, WS, WS, c), WS, h2, w2).transpose(0, 3, 1, 2)
    q = dwconv3(q, Wqdw1)
    kvw = window_partition(xds.transpose(0, 2, 3, 1), WS).reshape(-1, N, c) @ Wkv1.T
    kv = window_reverse(kvw.reshape(-1, WS, WS, 2 * c), WS, h2, w2).transpose(0, 3, 1, 2)
    kv = dwconv3(kv, Wkvdw1)
    q = jnp.roll(q, (-SS, -SS), axis=(-2, -1))
    kv = jnp.roll(kv, (-SS, -SS), axis=(-2, -1))
    kvp = window_partition(kv.transpose(0, 2, 3, 1), WS).transpose(0, 3, 1, 2)
    qp = window_partition(q.transpose(0, 2, 3, 1), WS).transpose(0, 3, 1, 2)
    B_ = qp.shape[0]
    q = l2norm(qp.reshape(B_, heads, ch, N), -2)
    k = l2norm(jnp.repeat(kvp[:, :c], 4, axis=0).reshape(B_, heads, ch, N), -2)
    v = jnp.repeat(kvp[:, c:], 4, axis=0).reshape(B_, heads, ch, N)
    attn = jnp.einsum("bhcn,bhcm->bhnm", q, k) * temp1 + rpb[None]
    nW = mask.shape[0]
    attn = (attn.reshape(B_ // nW, nW, heads, N, N) + mask[None, :, None]).reshape(
        B_, heads, N, N
    )
    attn = jax.nn.softmax(attn, axis=-1)
    out1 = jnp.einsum("bhnm,bhcm->bhnc", attn, v)
    out1 = out1.transpose(0, 2, 1, 3).reshape(B_, N, c) @ Wproj1.T
    out1 = window_reverse(out1.reshape(B_, WS, WS, c), WS, h, w)
    out1 = jnp.roll(out1, (SS, SS), axis=(1, 2)).transpose(0, 3, 1, 2)
    return out1


_jit_cache = {}


def _get_jits():
    if "b0" in _jit_cache:
        return _jit_cache["b0"], _jit_cache["b1"], _jit_cache["m0"], _jit_cache["m1"]
    devs = jax.devices()[:8]
    mesh0 = Mesh(np.array(devs[:4]), ("b",))
    mesh1 = Mesh(np.array(devs[4:8]), ("b",))

    def sh(mesh, spec):
        return NamedSharding(mesh, spec)

    b0 = jax.jit(
        _branch0,
        in_shardings=(
            sh(mesh0, P("b")),
            sh(mesh0, P()),
            sh(mesh0, P()),
            sh(mesh0, P()),
            sh(mesh0, P()),
            sh(mesh0, P()),
            sh(mesh0, P()),
        ),
        out_shardings=sh(mesh0, P("b")),
    )
    b1 = jax.jit(
        _branch1,
        in_shardings=(
            sh(mesh1, P("b")),
            sh(mesh1, P()),
            sh(mesh1, P()),
            sh(mesh1, P()),
            sh(mesh1, P()),
            sh(mesh1, P()),
            sh(mesh1, P()),
            sh(mesh1, P()),
            sh(mesh1, P()),
            sh(mesh1, P()),
        ),
        out_shardings=sh(mesh1, P("b")),
    )
    _jit_cache.update(b0=b0, b1=b1, m0=mesh0, m1=mesh1)
    return b0, b1, mesh0, mesh1


def kernel(**inputs):
    b0, b1, mesh0, mesh1 = _get_jits()
    x = np.ascontiguousarray(inputs["x"], dtype=np.float32)

    rpb_np = None
    idx = rel_pos_index(WS)
    rpb_np = inputs["rpb_table"][idx.reshape(-1)].reshape(
        WS * WS, WS * WS, HEADS
    ).transpose(2, 0, 1)
    mask_np = np.repeat(calc_mask_np(H // 2, W // 2, WS, SS), 4, axis=0)

    def put(mesh, arr, spec):
        return jax.device_put(np.asarray(arr), NamedSharding(mesh, spec))

    x0 = put(mesh0, x, P("b"))
    x1 = put(mesh1, x, P("b"))
    a0 = b0(
        x0,
        put(mesh0, inputs["Wq0"], P()),
        put(mesh0, inputs["Wqdw0"], P()),
        put(mesh0, inputs["Wkv0"], P()),
        put(mesh0, inputs["Wkvdw0"], P()),
        put(mesh0, inputs["Wproj0"], P()),
        put(mesh0, inputs["temp0"], P()),
    )
    a1 = b1(
        x1,
        put(mesh1, inputs["Wq1"], P()),
        put(mesh1, inputs["Wqdw1"], P()),
        put(mesh1, inputs["Wkv1"], P()),
        put(mesh1, inputs["Wkvdw1"], P()),
        put(mesh1, inputs["Wproj1"], P()),
        put(mesh1, inputs["temp1"], P()),
        put(mesh1, rpb_np, P()),
        put(mesh1, mask_np, P()),
        put(mesh1, inputs["Wds"], P()),
    )
    out = np.asarray(a0) + np.asarray(a1)
    return out.astype(np.float32)


def bench(inputs, iters=3):
    """Time the two branch executions with device-resident inputs."""
    import time

    b0, b1, mesh0, mesh1 = _get_jits()
    idx = rel_pos_index(WS)
    rpb_np = inputs["rpb_table"][idx.reshape(-1)].reshape(
        WS * WS, WS * WS, HEADS
    ).transpose(2, 0, 1)
    mask_np = np.repeat(calc_mask_np(H // 2, W // 2, WS, SS), 4, axis=0)

    def put(mesh, arr, spec):
        return jax.device_put(np.asarray(arr), NamedSharding(mesh, spec))

    a0_args = (
        put(mesh0, inputs["x"], P("b")),
        put(mesh0, inputs["Wq0"], P()),
        put(mesh0, inputs["Wqdw0"], P()),
        put(mesh0, inputs["Wkv0"], P()),
        put(mesh0, inputs["Wkvdw0"], P()),
        put(mesh0, inputs["Wproj0"], P()),
        put(mesh0, inputs["temp0"], P()),
    )
    a1_args = (
        put(mesh1, inputs["x"], P("b")),
        put(mesh1, inputs["Wq1"], P()),
        put(mesh1, inputs["Wqdw1"], P()),
        put(mesh1, inputs["Wkv1"], P()),
        put(mesh1, inputs["Wkvdw1"], P()),
        put(mesh1, inputs["Wproj1"], P()),
        put(mesh1, inputs["temp1"], P()),
        put(mesh1, rpb_np, P()),
        put(mesh1, mask_np, P()),
        put(mesh1, inputs["Wds"], P()),
    )
    # warm
    r0 = b0(*a0_args)
    r1 = b1(*a1_args)
    r0.block_until_ready()
    r1.block_until_ready()
    best = 1e30
    for _ in range(iters):
        t0 = time.time()
        r0 = b0(*a0_args)
        r1 = b1(*a1_args)
        r0.block_until_ready()
        r1.block_until_ready()
        best = min(best, time.time() - t0)
    return best


if __name__ == "__main__":
    # quick self-smoke with random inputs of the right shapes
    rng = np.random.default_rng(0)
    ins = dict(
        x=rng.standard_normal((B, DIM, H, W), dtype=np.float32),
        Wq0=rng.standard_normal((DIM, DIM), dtype=np.float32) * 0.02,
        Wqdw0=rng.standard_normal((DIM, 1, 3, 3), dtype=np.float32) * 0.02,
        Wkv0=rng.standard_normal((2 * DIM, DIM), dtype=np.float32) * 0.02,
        Wkvdw0=rng.standard_normal((2 * DIM, 1, 3, 3), dtype=np.float32) * 0.02,
        Wq1=rng.standard_normal((DIM, DIM), dtype=np.float32) * 0.02,
        Wqdw1=rng.standard_normal((DIM, 1, 3, 3), dtype=np.float32) * 0.02,
        Wkv1=rng.standard_normal((2 * DIM, DIM), dtype=np.float32) * 0.02,
        Wkvdw1=rng.standard_normal((2 * DIM, 1, 3, 3), dtype=np.float32) * 0.02,
        Wproj0=rng.standard_normal((DIM, DIM), dtype=np.float32) * 0.02,
        Wproj1=rng.standard_normal((DIM, DIM), dtype=np.float32) * 0.02,
        temp0=np.ones((HEADS, 1, 1), np.float32),
        temp1=np.ones((HEADS, 1, 1), np.float32),
        rpb_table=rng.standard_normal(((2 * WS - 1) ** 2, HEADS), dtype=np.float32) * 0.02,
        Wds=rng.standard_normal((DIM, DIM), dtype=np.float32) * 0.02,
    )
    out = kernel(**ins)
    print("out", out.shape, out.dtype, float(np.abs(out).max()))


# revision 4
# speedup vs baseline: 1.0031x; 1.0031x over previous
"""Trainium2 kernel for nn_Attention_57595511439927 (sparse_attention).

Sharding: 8 NeuronCores = 4 images x 2 branches.
  - devices 0-3: branch 0 (global channel/transposed attention) data-parallel over b
  - devices 4-7: branch 1 (shifted-window cosine attention)      data-parallel over b
Both branches are independent per image, so no collectives are needed.
Host does only slicing / reassembly / final add glue.
"""

import numpy as np
import jax
import jax.numpy as jnp
from jax.sharding import Mesh, NamedSharding, PartitionSpec as P

WS = 8
SS = WS // 2
DIM, HEADS = 192, 6
B, H, W = 4, 256, 256


def window_partition(x, ws):
    b, h, w, c = x.shape
    x = x.reshape(b, h // ws, ws, w // ws, ws, c)
    return x.transpose(0, 1, 3, 2, 4, 5).reshape(-1, ws, ws, c)


def window_reverse(win, ws, h, w):
    b = win.shape[0] // ((h // ws) * (w // ws))
    x = win.reshape(b, h // ws, w // ws, ws, ws, -1)
    return x.transpose(0, 1, 3, 2, 4, 5).reshape(b, h, w, -1)


def rel_pos_index(ws):
    coords = np.stack(np.meshgrid(np.arange(ws), np.arange(ws), indexing="ij"))
    cf = coords.reshape(2, -1)
    rel = (cf[:, :, None] - cf[:, None, :]).transpose(1, 2, 0)
    rel[:, :, 0] += ws - 1
    rel[:, :, 1] += ws - 1
    rel[:, :, 0] *= 2 * ws - 1
    return rel.sum(-1)


def calc_mask_np(h, w, ws, ss):
    img = np.zeros((1, h, w, 1), np.float32)
    cnt = 0
    sl = (slice(0, -ws), slice(-ws, -ss), slice(-ss, None))
    for hs in sl:
        for wsl in sl:
            img[:, hs, wsl, :] = cnt
            cnt += 1
    mw = (
        img.reshape(1, h // ws, ws, w // ws, ws, 1)
        .transpose(0, 1, 3, 2, 4, 5)
        .reshape(-1, ws * ws)
    )
    diff = mw[:, None, :] - mw[:, :, None]
    return np.where(diff != 0, -100.0, 0.0).astype(np.float32)


def conv1x1(x, w):
    return jnp.einsum("bchw,oc->bohw", x, w)


def dwconv3(x, w):
    # depthwise 3x3 stride 1 pad 1 implemented as 9 shifted adds (XLA friendly)
    b, c, h, ww = x.shape
    xp = jnp.pad(x, ((0, 0), (0, 0), (1, 1), (1, 1)))
    out = jnp.zeros_like(x)
    for i in range(3):
        for j in range(3):
            out = out + w[:, 0, i, j][None, :, None, None] * xp[:, :, i : i + h, j : j + ww]
    return out


def l2norm(x, axis):
    return x / jnp.maximum(jnp.linalg.norm(x, axis=axis, keepdims=True), 1e-12)


def _branch0(x, Wq0, Wqdw0, Wkv0, Wkvdw0, Wproj0, temp0):
    b, c, h, w = x.shape
    heads = HEADS
    ch = c // heads
    bf = jnp.bfloat16
    f32 = jnp.float32
    xb = x.astype(bf)
    q = dwconv3(conv1x1(xb, Wq0.astype(bf)), Wqdw0.astype(bf))
    kv = dwconv3(conv1x1(xb, Wkv0.astype(bf)), Wkvdw0.astype(bf))
    q = l2norm(q.reshape(b, heads, ch, h * w).astype(f32), -1)
    k = l2norm(kv[:, :c].reshape(b, heads, ch, h * w).astype(f32), -1)
    v = kv[:, c:].reshape(b, heads, ch, h * w)
    attn = jax.nn.softmax(
        jnp.einsum("bhcn,bhdn->bhcd", q.astype(bf), k.astype(bf),
                   preferred_element_type=f32) * temp0, axis=-1)
    out0 = jnp.einsum("bhcd,bhdn->bhcn", attn.astype(bf), v,
                      preferred_element_type=f32)
    out0 = out0.transpose(0, 3, 1, 2).reshape(b, h * w, c)
    out0 = jnp.einsum("npc,oc->npo", out0.astype(bf).reshape(b, h * w, c),
                      Wproj0.astype(bf), preferred_element_type=f32)
    return out0.reshape(b, h, w, c).transpose(0, 3, 1, 2)


def _branch1(x, Wq1, Wqdw1, Wkv1, Wkvdw1, Wproj1, temp1, rpb, mask, Wds):
    b, c, h, w = x.shape
    heads = HEADS
    ch = c // heads
    N = WS * WS
    bf = jnp.bfloat16
    f32 = jnp.float32
    h2, w2 = h // 2, w // 2
    xb = x.astype(bf)
    xds = x.reshape(b, c, h2, 2, w2, 2).mean(axis=(3, 5)).astype(bf)
    xds = xds + conv1x1(xds, Wds.astype(bf))
    qw = window_partition(xb.transpose(0, 2, 3, 1), WS).reshape(-1, N, c)
    qw = jnp.einsum("wnc,oc->wno", qw, Wq1.astype(bf), preferred_element_type=bf)
    q = window_reverse(qw.reshape(-1, WS, WS, c), WS, h2, w2).transpose(0, 3, 1, 2)
    q = dwconv3(q, Wqdw1.astype(bf))
    kvw = window_partition(xds.transpose(0, 2, 3, 1), WS).reshape(-1, N, c)
    kvw = jnp.einsum("wnc,oc->wno", kvw, Wkv1.astype(bf), preferred_element_type=bf)
    kv = window_reverse(kvw.reshape(-1, WS, WS, 2 * c), WS, h2, w2).transpose(0, 3, 1, 2)
    kv = dwconv3(kv, Wkvdw1.astype(bf))
    q = jnp.roll(q, (-SS, -SS), axis=(-2, -1))
    kv = jnp.roll(kv, (-SS, -SS), axis=(-2, -1))
    kvp = window_partition(kv.transpose(0, 2, 3, 1), WS).transpose(0, 3, 1, 2)
    qp = window_partition(q.transpose(0, 2, 3, 1), WS).transpose(0, 3, 1, 2)
    B_ = qp.shape[0]
    q = l2norm(qp.reshape(B_, heads, ch, N).astype(f32), -2)
    k = l2norm(jnp.repeat(kvp[:, :c], 4, axis=0).reshape(B_, heads, ch, N).astype(f32), -2)
    v = jnp.repeat(kvp[:, c:], 4, axis=0).reshape(B_, heads, ch, N)
    attn = jnp.einsum("bhcn,bhcm->bhnm", q.astype(bf), k.astype(bf),
                      preferred_element_type=f32) * temp1 + rpb[None]
    nW = mask.shape[0]
    attn = (attn.reshape(B_ // nW, nW, heads, N, N) + mask[None, :, None]).reshape(
        B_, heads, N, N
    )
    attn = jax.nn.softmax(attn, axis=-1)
    out1 = jnp.einsum("bhnm,bhcm->bhnc", attn.astype(bf), v,
                      preferred_element_type=f32)
    out1 = out1.transpose(0, 2, 1, 3).reshape(B_, N, c)
    out1 = jnp.einsum("wnc,oc->wno", out1.astype(bf), Wproj1.astype(bf),
                      preferred_element_type=f32)
    out1 = window_reverse(out1.reshape(B_, WS, WS, c), WS, h, w)
    out1 = jnp.roll(out1, (SS, SS), axis=(1, 2)).transpose(0, 3, 1, 2)
    return out1


_jit_cache = {}


def _get_jits():
    if "b0" in _jit_cache:
        return _jit_cache["b0"], _jit_cache["b1"], _jit_cache["m0"], _jit_cache["m1"]
    devs = jax.devices()[:8]
    mesh0 = Mesh(np.array(devs[:4]), ("b",))
    mesh1 = Mesh(np.array(devs[4:8]), ("b",))

    def sh(mesh, spec):
        return NamedSharding(mesh, spec)

    b0 = jax.jit(
        _branch0,
        in_shardings=(
            sh(mesh0, P("b")),
            sh(mesh0, P()),
            sh(mesh0, P()),
            sh(mesh0, P()),
            sh(mesh0, P()),
            sh(mesh0, P()),
            sh(mesh0, P()),
        ),
        out_shardings=sh(mesh0, P("b")),
    )
    b1 = jax.jit(
        _branch1,
        in_shardings=(
            sh(mesh1, P("b")),
            sh(mesh1, P()),
            sh(mesh1, P()),
            sh(mesh1, P()),
            sh(mesh1, P()),
            sh(mesh1, P()),
            sh(mesh1, P()),
            sh(mesh1, P()),
            sh(mesh1, P()),
            sh(mesh1, P()),
        ),
        out_shardings=sh(mesh1, P("b")),
    )
    _jit_cache.update(b0=b0, b1=b1, m0=mesh0, m1=mesh1)
    return b0, b1, mesh0, mesh1


def kernel(**inputs):
    b0, b1, mesh0, mesh1 = _get_jits()
    x = np.ascontiguousarray(inputs["x"], dtype=np.float32)

    rpb_np = None
    idx = rel_pos_index(WS)
    rpb_np = inputs["rpb_table"][idx.reshape(-1)].reshape(
        WS * WS, WS * WS, HEADS
    ).transpose(2, 0, 1)
    mask_np = np.repeat(calc_mask_np(H // 2, W // 2, WS, SS), 4, axis=0)

    def put(mesh, arr, spec):
        return jax.device_put(np.asarray(arr), NamedSharding(mesh, spec))

    x0 = put(mesh0, x, P("b"))
    x1 = put(mesh1, x, P("b"))
    a0 = b0(
        x0,
        put(mesh0, inputs["Wq0"], P()),
        put(mesh0, inputs["Wqdw0"], P()),
        put(mesh0, inputs["Wkv0"], P()),
        put(mesh0, inputs["Wkvdw0"], P()),
        put(mesh0, inputs["Wproj0"], P()),
        put(mesh0, inputs["temp0"], P()),
    )
    a1 = b1(
        x1,
        put(mesh1, inputs["Wq1"], P()),
        put(mesh1, inputs["Wqdw1"], P()),
        put(mesh1, inputs["Wkv1"], P()),
        put(mesh1, inputs["Wkvdw1"], P()),
        put(mesh1, inputs["Wproj1"], P()),
        put(mesh1, inputs["temp1"], P()),
        put(mesh1, rpb_np, P()),
        put(mesh1, mask_np, P()),
        put(mesh1, inputs["Wds"], P()),
    )
    out = np.asarray(a0) + np.asarray(a1)
    return out.astype(np.float32)


def bench(inputs, iters=3):
    """Time the two branch executions with device-resident inputs."""
    import time

    b0, b1, mesh0, mesh1 = _get_jits()
    idx = rel_pos_index(WS)
    rpb_np = inputs["rpb_table"][idx.reshape(-1)].reshape(
        WS * WS, WS * WS, HEADS
    ).transpose(2, 0, 1)
    mask_np = np.repeat(calc_mask_np(H // 2, W // 2, WS, SS), 4, axis=0)

    def put(mesh, arr, spec):
        return jax.device_put(np.asarray(arr), NamedSharding(mesh, spec))

    a0_args = (
        put(mesh0, inputs["x"], P("b")),
        put(mesh0, inputs["Wq0"], P()),
        put(mesh0, inputs["Wqdw0"], P()),
        put(mesh0, inputs["Wkv0"], P()),
        put(mesh0, inputs["Wkvdw0"], P()),
        put(mesh0, inputs["Wproj0"], P()),
        put(mesh0, inputs["temp0"], P()),
    )
    a1_args = (
        put(mesh1, inputs["x"], P("b")),
        put(mesh1, inputs["Wq1"], P()),
        put(mesh1, inputs["Wqdw1"], P()),
        put(mesh1, inputs["Wkv1"], P()),
        put(mesh1, inputs["Wkvdw1"], P()),
        put(mesh1, inputs["Wproj1"], P()),
        put(mesh1, inputs["temp1"], P()),
        put(mesh1, rpb_np, P()),
        put(mesh1, mask_np, P()),
        put(mesh1, inputs["Wds"], P()),
    )
    # warm
    r0 = b0(*a0_args)
    r1 = b1(*a1_args)
    r0.block_until_ready()
    r1.block_until_ready()
    best = 1e30
    for _ in range(iters):
        t0 = time.time()
        r0 = b0(*a0_args)
        r1 = b1(*a1_args)
        r0.block_until_ready()
        r1.block_until_ready()
        best = min(best, time.time() - t0)
    return best


if __name__ == "__main__":
    # quick self-smoke with random inputs of the right shapes
    rng = np.random.default_rng(0)
    ins = dict(
        x=rng.standard_normal((B, DIM, H, W), dtype=np.float32),
        Wq0=rng.standard_normal((DIM, DIM), dtype=np.float32) * 0.02,
        Wqdw0=rng.standard_normal((DIM, 1, 3, 3), dtype=np.float32) * 0.02,
        Wkv0=rng.standard_normal((2 * DIM, DIM), dtype=np.float32) * 0.02,
        Wkvdw0=rng.standard_normal((2 * DIM, 1, 3, 3), dtype=np.float32) * 0.02,
        Wq1=rng.standard_normal((DIM, DIM), dtype=np.float32) * 0.02,
        Wqdw1=rng.standard_normal((DIM, 1, 3, 3), dtype=np.float32) * 0.02,
        Wkv1=rng.standard_normal((2 * DIM, DIM), dtype=np.float32) * 0.02,
        Wkvdw1=rng.standard_normal((2 * DIM, 1, 3, 3), dtype=np.float32) * 0.02,
        Wproj0=rng.standard_normal((DIM, DIM), dtype=np.float32) * 0.02,
        Wproj1=rng.standard_normal((DIM, DIM), dtype=np.float32) * 0.02,
        temp0=np.ones((HEADS, 1, 1), np.float32),
        temp1=np.ones((HEADS, 1, 1), np.float32),
        rpb_table=rng.standard_normal(((2 * WS - 1) ** 2, HEADS), dtype=np.float32) * 0.02,
        Wds=rng.standard_normal((DIM, DIM), dtype=np.float32) * 0.02,
    )
    out = kernel(**ins)
    print("out", out.shape, out.dtype, float(np.abs(out).max()))


# revision 6
# speedup vs baseline: 1.6131x; 1.6082x over previous
"""Trainium2 kernel for nn_Attention_57595511439927 (sparse_attention).

Sharding: 8 NeuronCores = 4 images x 2 branches.
  - devices 0-3: branch 0 (global channel/transposed attention) data-parallel over b
  - devices 4-7: branch 1 (shifted-window cosine attention)      data-parallel over b
Both branches are independent per image, so no collectives are needed.
Host does only slicing / reassembly / final add glue.
"""

import numpy as np
import jax
import jax.numpy as jnp
from jax.sharding import Mesh, NamedSharding, PartitionSpec as P

WS = 8
SS = WS // 2
DIM, HEADS = 192, 6
B, H, W = 4, 256, 256


def window_partition(x, ws):
    b, h, w, c = x.shape
    x = x.reshape(b, h // ws, ws, w // ws, ws, c)
    return x.transpose(0, 1, 3, 2, 4, 5).reshape(-1, ws, ws, c)


def window_reverse(win, ws, h, w):
    b = win.shape[0] // ((h // ws) * (w // ws))
    x = win.reshape(b, h // ws, w // ws, ws, ws, -1)
    return x.transpose(0, 1, 3, 2, 4, 5).reshape(b, h, w, -1)


def rel_pos_index(ws):
    coords = np.stack(np.meshgrid(np.arange(ws), np.arange(ws), indexing="ij"))
    cf = coords.reshape(2, -1)
    rel = (cf[:, :, None] - cf[:, None, :]).transpose(1, 2, 0)
    rel[:, :, 0] += ws - 1
    rel[:, :, 1] += ws - 1
    rel[:, :, 0] *= 2 * ws - 1
    return rel.sum(-1)


def calc_mask_np(h, w, ws, ss):
    img = np.zeros((1, h, w, 1), np.float32)
    cnt = 0
    sl = (slice(0, -ws), slice(-ws, -ss), slice(-ss, None))
    for hs in sl:
        for wsl in sl:
            img[:, hs, wsl, :] = cnt
            cnt += 1
    mw = (
        img.reshape(1, h // ws, ws, w // ws, ws, 1)
        .transpose(0, 1, 3, 2, 4, 5)
        .reshape(-1, ws * ws)
    )
    diff = mw[:, None, :] - mw[:, :, None]
    return np.where(diff != 0, -100.0, 0.0).astype(np.float32)


def conv1x1(x, w):
    return jnp.einsum("bchw,oc->bohw", x, w)


def dwconv3(x, w):
    # depthwise 3x3 stride 1 pad 1 implemented as 9 shifted adds (XLA friendly)
    b, c, h, ww = x.shape
    xp = jnp.pad(x, ((0, 0), (0, 0), (1, 1), (1, 1)))
    out = jnp.zeros_like(x)
    for i in range(3):
        for j in range(3):
            out = out + w[:, 0, i, j][None, :, None, None] * xp[:, :, i : i + h, j : j + ww]
    return out


def l2norm(x, axis):
    return x / jnp.maximum(jnp.linalg.norm(x, axis=axis, keepdims=True), 1e-12)


def _branch0(x, Wq0, Wqdw0, Wkv0, Wkvdw0, Wproj0, temp0):
    b, c, h, w = x.shape
    heads = HEADS
    ch = c // heads
    bf = jnp.bfloat16
    f32 = jnp.float32
    xb = x.astype(bf)
    q = dwconv3(conv1x1(xb, Wq0.astype(bf)), Wqdw0.astype(bf))
    kv = dwconv3(conv1x1(xb, Wkv0.astype(bf)), Wkvdw0.astype(bf))
    q = l2norm(q.reshape(b, heads, ch, h * w).astype(f32), -1)
    k = l2norm(kv[:, :c].reshape(b, heads, ch, h * w).astype(f32), -1)
    v = kv[:, c:].reshape(b, heads, ch, h * w)
    attn = jax.nn.softmax(
        jnp.einsum("bhcn,bhdn->bhcd", q.astype(bf), k.astype(bf),
                   preferred_element_type=f32) * temp0, axis=-1)
    out0 = jnp.einsum("bhcd,bhdn->bhcn", attn.astype(bf), v,
                      preferred_element_type=f32)
    out0 = out0.transpose(0, 3, 1, 2).reshape(b, h * w, c)
    out0 = jnp.einsum("npc,oc->npo", out0.astype(bf).reshape(b, h * w, c),
                      Wproj0.astype(bf), preferred_element_type=f32)
    return out0.reshape(b, h, w, c).transpose(0, 3, 1, 2)


def _branch1(x, Wq1, Wqdw1, Wkv1, Wkvdw1, Wproj1, temp1, rpb, mask, Wds):
    b, c, h, w = x.shape
    heads = HEADS
    ch = c // heads
    N = WS * WS
    bf = jnp.bfloat16
    f32 = jnp.float32
    h2, w2 = h // 2, w // 2
    xb = x.astype(bf)
    xds = x.reshape(b, c, h2, 2, w2, 2).mean(axis=(3, 5)).astype(bf)
    xds = xds + conv1x1(xds, Wds.astype(bf))
    qw = window_partition(xb.transpose(0, 2, 3, 1), WS).reshape(-1, N, c)
    qw = jnp.einsum("wnc,oc->wno", qw, Wq1.astype(bf), preferred_element_type=bf)
    q = window_reverse(qw.reshape(-1, WS, WS, c), WS, h2, w2).transpose(0, 3, 1, 2)
    q = dwconv3(q, Wqdw1.astype(bf))
    kvw = window_partition(xds.transpose(0, 2, 3, 1), WS).reshape(-1, N, c)
    kvw = jnp.einsum("wnc,oc->wno", kvw, Wkv1.astype(bf), preferred_element_type=bf)
    kv = window_reverse(kvw.reshape(-1, WS, WS, 2 * c), WS, h2, w2).transpose(0, 3, 1, 2)
    kv = dwconv3(kv, Wkvdw1.astype(bf))
    q = jnp.roll(q, (-SS, -SS), axis=(-2, -1))
    kv = jnp.roll(kv, (-SS, -SS), axis=(-2, -1))
    kvp = window_partition(kv.transpose(0, 2, 3, 1), WS).transpose(0, 3, 1, 2)
    qp = window_partition(q.transpose(0, 2, 3, 1), WS).transpose(0, 3, 1, 2)
    B_ = qp.shape[0]
    q = l2norm(qp.reshape(B_, heads, ch, N).astype(f32), -2)
    k = l2norm(jnp.repeat(kvp[:, :c], 4, axis=0).reshape(B_, heads, ch, N).astype(f32), -2)
    v = jnp.repeat(kvp[:, c:], 4, axis=0).reshape(B_, heads, ch, N)
    attn = jnp.einsum("bhcn,bhcm->bhnm", q.astype(bf), k.astype(bf),
                      preferred_element_type=f32) * temp1 + rpb[None]
    nW = mask.shape[0]
    attn = (attn.reshape(B_ // nW, nW, heads, N, N) + mask[None, :, None]).reshape(
        B_, heads, N, N
    )
    attn = jax.nn.softmax(attn, axis=-1)
    out1 = jnp.einsum("bhnm,bhcm->bhnc", attn.astype(bf), v,
                      preferred_element_type=f32)
    out1 = out1.transpose(0, 2, 1, 3).reshape(B_, N, c)
    out1 = jnp.einsum("wnc,oc->wno", out1.astype(bf), Wproj1.astype(bf),
                      preferred_element_type=f32)
    out1 = window_reverse(out1.reshape(B_, WS, WS, c), WS, h, w)
    out1 = jnp.roll(out1, (SS, SS), axis=(1, 2)).transpose(0, 3, 1, 2)
    return out1


_jit_cache = {}


def _get_jits():
    if "b0" in _jit_cache:
        return _jit_cache["b0"], _jit_cache["b1"], _jit_cache["m0"], _jit_cache["m1"]
    devs = jax.devices()[:8]
    mesh0 = Mesh(np.array(devs[:4]), ("b",))
    mesh1 = Mesh(np.array(devs[4:8]), ("b",))

    def sh(mesh, spec):
        return NamedSharding(mesh, spec)

    b0 = jax.jit(
        _branch0,
        in_shardings=(
            sh(mesh0, P("b")),
            sh(mesh0, P()),
            sh(mesh0, P()),
            sh(mesh0, P()),
            sh(mesh0, P()),
            sh(mesh0, P()),
            sh(mesh0, P()),
        ),
        out_shardings=sh(mesh0, P("b")),
    )
    b1 = jax.jit(
        _branch1,
        in_shardings=(
            sh(mesh1, P("b")),
            sh(mesh1, P()),
            sh(mesh1, P()),
            sh(mesh1, P()),
            sh(mesh1, P()),
            sh(mesh1, P()),
            sh(mesh1, P()),
            sh(mesh1, P()),
            sh(mesh1, P()),
            sh(mesh1, P()),
        ),
        out_shardings=sh(mesh1, P("b")),
    )
    _jit_cache.update(b0=b0, b1=b1, m0=mesh0, m1=mesh1)
    return b0, b1, mesh0, mesh1


def kernel(**inputs):
    b0, b1, mesh0, mesh1 = _get_jits()
    x = np.ascontiguousarray(inputs["x"], dtype=np.float32)

    rpb_np = None
    idx = rel_pos_index(WS)
    rpb_np = inputs["rpb_table"][idx.reshape(-1)].reshape(
        WS * WS, WS * WS, HEADS
    ).transpose(2, 0, 1)
    mask_np = np.repeat(calc_mask_np(H // 2, W // 2, WS, SS), 4, axis=0)

    def put(mesh, arr, spec):
        return jax.device_put(np.asarray(arr), NamedSharding(mesh, spec))

    x0 = put(mesh0, x, P("b"))
    x1 = put(mesh1, x, P("b"))
    a0_args = (
        x0,
        put(mesh0, inputs["Wq0"], P()),
        put(mesh0, inputs["Wqdw0"], P()),
        put(mesh0, inputs["Wkv0"], P()),
        put(mesh0, inputs["Wkvdw0"], P()),
        put(mesh0, inputs["Wproj0"], P()),
        put(mesh0, inputs["temp0"], P()),
    )
    a1_args = (
        x1,
        put(mesh1, inputs["Wq1"], P()),
        put(mesh1, inputs["Wqdw1"], P()),
        put(mesh1, inputs["Wkv1"], P()),
        put(mesh1, inputs["Wkvdw1"], P()),
        put(mesh1, inputs["Wproj1"], P()),
        put(mesh1, inputs["temp1"], P()),
        put(mesh1, rpb_np, P()),
        put(mesh1, mask_np, P()),
        put(mesh1, inputs["Wds"], P()),
    )
    import threading

    res = [None, None]

    def _r0():
        res[0] = b0(*a0_args)
        res[0].block_until_ready()

    def _r1():
        res[1] = b1(*a1_args)
        res[1].block_until_ready()

    th0 = threading.Thread(target=_r0)
    th1 = threading.Thread(target=_r1)
    th0.start()
    th1.start()
    th0.join()
    th1.join()
    out = np.asarray(res[0]) + np.asarray(res[1])
    return out.astype(np.float32)


def bench(inputs, iters=3):
    """Time the two branch executions with device-resident inputs."""
    import time

    b0, b1, mesh0, mesh1 = _get_jits()
    idx = rel_pos_index(WS)
    rpb_np = inputs["rpb_table"][idx.reshape(-1)].reshape(
        WS * WS, WS * WS, HEADS
    ).transpose(2, 0, 1)
    mask_np = np.repeat(calc_mask_np(H // 2, W // 2, WS, SS), 4, axis=0)

    def put(mesh, arr, spec):
        return jax.device_put(np.asarray(arr), NamedSharding(mesh, spec))

    a0_args = (
        put(mesh0, inputs["x"], P("b")),
        put(mesh0, inputs["Wq0"], P()),
        put(mesh0, inputs["Wqdw0"], P()),
        put(mesh0, inputs["Wkv0"], P()),
        put(mesh0, inputs["Wkvdw0"], P()),
        put(mesh0, inputs["Wproj0"], P()),
        put(mesh0, inputs["temp0"], P()),
    )
    a1_args = (
        put(mesh1, inputs["x"], P("b")),
        put(mesh1, inputs["Wq1"], P()),
        put(mesh1, inputs["Wqdw1"], P()),
        put(mesh1, inputs["Wkv1"], P()),
        put(mesh1, inputs["Wkvdw1"], P()),
        put(mesh1, inputs["Wproj1"], P()),
        put(mesh1, inputs["temp1"], P()),
        put(mesh1, rpb_np, P()),
        put(mesh1, mask_np, P()),
        put(mesh1, inputs["Wds"], P()),
    )
    # warm
    r0 = b0(*a0_args)
    r1 = b1(*a1_args)
    r0.block_until_ready()
    r1.block_until_ready()
    import threading

    best = 1e30
    for _ in range(iters):
        t0 = time.time()
        th0 = threading.Thread(target=lambda: b0(*a0_args).block_until_ready())
        th1 = threading.Thread(target=lambda: b1(*a1_args).block_until_ready())
        th0.start()
        th1.start()
        th0.join()
        th1.join()
        best = min(best, time.time() - t0)
    return best


if __name__ == "__main__":
    # quick self-smoke with random inputs of the right shapes
    rng = np.random.default_rng(0)
    ins = dict(
        x=rng.standard_normal((B, DIM, H, W), dtype=np.float32),
        Wq0=rng.standard_normal((DIM, DIM), dtype=np.float32) * 0.02,
        Wqdw0=rng.standard_normal((DIM, 1, 3, 3), dtype=np.float32) * 0.02,
        Wkv0=rng.standard_normal((2 * DIM, DIM), dtype=np.float32) * 0.02,
        Wkvdw0=rng.standard_normal((2 * DIM, 1, 3, 3), dtype=np.float32) * 0.02,
        Wq1=rng.standard_normal((DIM, DIM), dtype=np.float32) * 0.02,
        Wqdw1=rng.standard_normal((DIM, 1, 3, 3), dtype=np.float32) * 0.02,
        Wkv1=rng.standard_normal((2 * DIM, DIM), dtype=np.float32) * 0.02,
        Wkvdw1=rng.standard_normal((2 * DIM, 1, 3, 3), dtype=np.float32) * 0.02,
        Wproj0=rng.standard_normal((DIM, DIM), dtype=np.float32) * 0.02,
        Wproj1=rng.standard_normal((DIM, DIM), dtype=np.float32) * 0.02,
        temp0=np.ones((HEADS, 1, 1), np.float32),
        temp1=np.ones((HEADS, 1, 1), np.float32),
        rpb_table=rng.standard_normal(((2 * WS - 1) ** 2, HEADS), dtype=np.float32) * 0.02,
        Wds=rng.standard_normal((DIM, DIM), dtype=np.float32) * 0.02,
    )
    out = kernel(**ins)
    print("out", out.shape, out.dtype, float(np.abs(out).max()))
